# revision 1
# baseline (speedup 1.0000x reference)
"""Trainium2 Bass kernel for nn_AttentionWithFastKANTransform (8 NeuronCores).

kernel(**inputs) takes the FULL unsharded inputs (as produced by
reference.setup_inputs()) and returns the full [128, 256] float32 output.

Distribution: S (=8192 keys) is sharded across the 8 cores (1024 each); q and
the q/g/k/v KAN parameters are replicated; o-side parameters are sharded over
output columns. Per-core softmax partials (unnormalized PV sums plus
sum-of-exp via a ones-column in the PV rhs; no max subtraction -- |logit|
stays < ~25, safe in fp32/bf16) are combined with a single 8-core AllReduce,
after which every core finishes the gate + output-FastKAN for its own
32-column output shard; the host concatenates the shards.

Host-side prep is layout/constant refactorization only: q/k/v are
pre-transposed to [dims, tokens] (k/v cast bf16 -- verified harmless), spline
weights reordered grid-major with C_j = exp(-grid_j^2/denom^2) folded in,
base weights halved (silu2 identity), all bf16.

On-device RBF basis: b'_j = basis_j / C_j satisfies
b'_0 = exp(-x(x+4)/D^2) and b'_4 = exp(-x(x-2g4)/D^2) (squares computed as
scalar_tensor_tensor products on DVE), b'_{j+1} = b'_j * exp(3.5x);
silu2(x) = (1 + tanh(x/2)) x; the gate uses g = sigmoid form
(1+tanh(z/2))/2 with v_bb folded past the softmax normalizer. Attention
logits are computed transposed [s, b] via block-diagonal per-head wq panels
(one 512-col matmul per (ch, s-chunk)); PV accumulates all 8 S-chunks into
one PSUM tile (explicit memset + start=False accumulation -- per-slice
start=True wipes the whole accumulation bank). The emission order JIT-feeds
the PE from a k-first DMA stream; the ACT function table is pre-loaded
outside the timing loop.
"""

import math
from contextlib import ExitStack

import numpy as np
import ml_dtypes

import concourse.bass as bass
import concourse.tile as tile
from concourse import bacc, mybir
from concourse.masks import make_identity

F32 = mybir.dt.float32
BF16 = mybir.dt.bfloat16
FP16 = mybir.dt.float16

P = 128
B = 128
S_C = 1024
DIN = 256
DOUT = 256
NG = 8
H = 8
HD = 32
NKC = 16
OSH = 32
GRID = np.linspace(-2.0, 2.0, NG)
DEN = (2.0 - (-2.0)) / (NG - 1)
NORM = 1.0 / math.sqrt(HD)
AR_GROUP = [list(range(8))]
RS = 1.0 / DEN ** 2          # 3.0625
G4X2 = 2.0 * GRID[4]         # 4/7

MM = mybir.AluOpType.mult
AD = mybir.AluOpType.add
AF = mybir.ActivationFunctionType


def build_program(mock_ar=False, num_devices=8, phase=4, loop_n=None, dbg=False):
    nc = bacc.Bacc("TRN2", target_bir_lowering=False, debug=False,
                   num_devices=num_devices)

    d_qT = nc.dram_tensor("q", [P, 2, B], F32, kind="ExternalInput").ap()
    d_kT = nc.dram_tensor("k", [P, 2, S_C], BF16, kind="ExternalInput").ap()
    d_vT = nc.dram_tensor("v", [P, 2, S_C], BF16, kind="ExternalInput").ap()
    dw = {}
    for nm, dout in (("q", DOUT), ("k", DOUT), ("v", DOUT), ("g", DOUT), ("o", OSH)):
        dw[f"{nm}_sw"] = nc.dram_tensor(f"{nm}_sw", [P, NKC, dout], BF16,
                                        kind="ExternalInput").ap()
        dw[f"{nm}_bw"] = nc.dram_tensor(f"{nm}_bw", [P, 2, dout], BF16,
                                        kind="ExternalInput").ap()
    d_qbb = nc.dram_tensor("q_bb", [P, 2], F32, kind="ExternalInput").ap()   # * NORM
    d_kbb = nc.dram_tensor("k_bb", [P, 2], F32, kind="ExternalInput").ap()
    d_gbb = nc.dram_tensor("g_bb", [1, DOUT], BF16, kind="ExternalInput").ap()
    d_vbb = nc.dram_tensor("v_bb", [P, DOUT], BF16, kind="ExternalInput").ap()
    d_obb = nc.dram_tensor("o_bb", [1, OSH], BF16, kind="ExternalInput").ap()
    d_out = nc.dram_tensor("out", [B, OSH], F32, kind="ExternalOutput").ap()

    cc_in = nc.dram_tensor("cc_in", [B, H, HD + 1], F32).ap()
    cc_out = nc.dram_tensor("cc_out", [B, H, HD + 1], F32, addr_space="Shared").ap()

    with ExitStack() as ctx:
        tc = ctx.enter_context(tile.TileContext(nc))
        cons = ctx.enter_context(tc.tile_pool(name="cons", bufs=1))
        wpool = ctx.enter_context(tc.tile_pool(name="wpool", bufs=1))
        xpool = ctx.enter_context(tc.tile_pool(name="xpool", bufs=1))
        cpool = ctx.enter_context(tc.tile_pool(name="cpool", bufs=2))
        spool = ctx.enter_context(tc.tile_pool(name="spool", bufs=2))
        attn = ctx.enter_context(tc.tile_pool(name="attn", bufs=1))
        psA = ctx.enter_context(tc.tile_pool(name="psA", bufs=2, space="PSUM"))
        psL = ctx.enter_context(tc.tile_pool(name="psL", bufs=2, space="PSUM"))
        psC = ctx.enter_context(tc.tile_pool(name="psC", bufs=2, space="PSUM"))

        ident = cons.tile([P, P], F32)
        make_identity(nc, ident)
        ones_b = cons.tile([1, P], BF16)
        nc.vector.memset(ones_b, 1.0)
        # Pre-load the ACT function table (Exp/Tanh/Identity set) outside the
        # loop so per-iteration activations never reload it.
        act_warm = cons.tile([1, P], BF16)
        nc.scalar.activation(act_warm, ones_b, AF.Exp)
        _cbias = {}

        def cb(val):
            v = float(val)
            if v not in _cbias:
                tcb = cons.tile([P, 1], F32, name=f"cb_{len(_cbias)}")
                nc.vector.memset(tcb, v)
                _cbias[v] = tcb
            return _cbias[v]
        wq4 = cons.tile([P, 2, 4, B], FP16)
        nc.gpsimd.memset(wq4, 0.0)
        wv_sb = cons.tile([P, 8, H, HD + 1], BF16)
        nc.gpsimd.memset(wv_sb[:, :, :, HD:HD + 1], 1.0)

        loop_cm = tc.For_i(0, loop_n, 1) if loop_n else None
        if loop_cm:
            loop_cm.__enter__()

        # ---- DMA stream (JIT order: k path first, tiny tensors later) ----
        kT = xpool.tile([P, 2, S_C], BF16, name="kT")
        nc.sync.dma_start(out=kT[:, :, 0:512], in_=d_kT[:, :, 0:512])
        wsp = {}

        def load_w(nm, dout, bw_first=False, split=False):
            def sw():
                t = wpool.tile([P, NKC, dout], BF16, name=f"{nm}sw")
                if split:
                    nc.sync.dma_start(out=t[:, NKC // 2:, :],
                                      in_=dw[f"{nm}_sw"][:, NKC // 2:, :])
                    nc.sync.dma_start(out=t[:, 0:NKC // 2, :],
                                      in_=dw[f"{nm}_sw"][:, 0:NKC // 2, :])
                else:
                    nc.sync.dma_start(out=t, in_=dw[f"{nm}_sw"])
                wsp[f"{nm}sw"] = t

            def bw():
                tb = wpool.tile([P, 2, dout], BF16, name=f"{nm}bw")
                nc.sync.dma_start(out=tb, in_=dw[f"{nm}_bw"])
                wsp[f"{nm}bw"] = tb

            if bw_first:
                bw(); sw()
            else:
                sw(); bw()

        load_w("k", DOUT, bw_first=True, split=True)
        nc.sync.dma_start(out=kT[:, :, 512:1024], in_=d_kT[:, :, 512:1024])
        kbb = cons.tile([P, 2], F32)
        nc.sync.dma_start(out=kbb, in_=d_kbb)
        vT = xpool.tile([P, 2, S_C], BF16, name="vT")
        nc.sync.dma_start(out=vT[:, :, 0:512], in_=d_vT[:, :, 0:512])
        qT = spool.tile([P, 2, B], F32, bufs=1)
        nc.sync.dma_start(out=qT, in_=d_qT)
        load_w("q", DOUT)
        qbbn = cons.tile([P, 2], F32)
        nc.sync.dma_start(out=qbbn, in_=d_qbb)
        load_w("v", DOUT)
        nc.sync.dma_start(out=vT[:, :, 512:1024], in_=d_vT[:, :, 512:1024])
        load_w("g", DOUT)
        gbb = cons.tile([1, DOUT], BF16)
        nc.sync.dma_start(out=gbb, in_=d_gbb)
        vbb = cons.tile([P, DOUT], BF16)
        nc.sync.dma_start(out=vbb, in_=d_vbb)
        load_w("o", OSH)
        obb = cons.tile([1, OSH], BF16)
        nc.sync.dma_start(out=obb, in_=d_obb)

        # kc consumption order matching basis readiness (anchors j=0,4 first,
        # then recurrence steps); kc = j*2+ch
        KC_ORDER = [8, 9, 10, 11, 12, 13, 14, 15, 0, 1, 2, 3, 4, 5, 6, 7]

        # ---- chain / silu helpers (chunked) ----
        def chain_chunk(x3, basis, sl, W, sq=None, chs=(0, 1), act_sq=False):
            """x3: [P, nch, W]; writes basis[:, j*2+ch, sl] bf16 for ch in chs."""
            sq = sq or nc.vector
            nch = len(chs)
            if nch == 2:
                def bsl(j):
                    return basis[:, j * 2:(j + 1) * 2, sl]
            else:
                def bsl(j):
                    kc = j * 2 + chs[0]
                    return basis[:, kc:kc + 1, sl]
            r = cpool.tile([P, 2, 512], BF16, name="ch_r", tag="ch_r")[:, 0:nch, 0:W]
            nc.scalar.activation(r, x3, AF.Exp, scale=2.0 / DEN)
            if act_sq:
                # anchors fully on ACT (tail: DVE is busy with the gate)
                for a in (4, 0):
                    u = cpool.tile([P, 2, 512], F32, name="ch_u",
                                   tag="ch_s")[:, 0:nch, 0:W]
                    nc.scalar.activation(u, x3, AF.Square, scale=1.0 / DEN,
                                         bias=cb(-GRID[a] / DEN))
                    nc.scalar.activation(bsl(a), u, AF.Exp, scale=-1.0,
                                         bias=cb(GRID[a] ** 2 / DEN ** 2))
            else:
                s = cpool.tile([P, 2, 512], F32, name="ch_s", tag="ch_s")[:, 0:nch, 0:W]
                sq.scalar_tensor_tensor(out=s, in0=x3, scalar=-G4X2, in1=x3,
                                        op0=AD, op1=MM)
                t = cpool.tile([P, 2, 512], F32, name="ch_t", tag="ch_t")[:, 0:nch, 0:W]
                sq.scalar_tensor_tensor(out=t, in0=x3, scalar=4.0, in1=x3,
                                        op0=AD, op1=MM)
                nc.scalar.activation(bsl(4), s, AF.Exp, scale=-RS)
                nc.scalar.activation(bsl(0), t, AF.Exp, scale=-RS)
            for j in (5, 1, 6, 2, 7, 3):
                nc.vector.tensor_tensor(out=bsl(j), in0=bsl(j - 1), in1=r, op=MM)

        def silu_chunk(x3, out3, W):
            """out = (1 + tanh(x/2)) * x = 2*silu(x), bf16 (bw pre-halved)."""
            nch = x3.shape[1]
            t = cpool.tile([P, 2, 512], BF16, name="si_t", tag="si_t")[:, 0:nch, 0:W]
            nc.scalar.activation(t, x3, AF.Tanh, scale=0.5)
            nc.vector.scalar_tensor_tensor(out=out3, in0=t, scalar=1.0, in1=x3,
                                           op0=AD, op1=MM)

        # ---- k chain half A ----
        kbasis = xpool.tile([P, NKC, S_C], BF16, name="kbasis")
        ksilu = xpool.tile([P, 2, S_C], BF16, name="ksilu")
        chain_chunk(kT[:, :, 0:512], kbasis, slice(0, 512), 512)
        silu_chunk(kT[:, :, 0:512], ksilu[:, :, 0:512], 512)

        # ---- wk spline (transposed out, fp16) ----
        wkT = attn.tile([P, 2, S_C], FP16, bufs=2)

        def wk_part(dh, sl):
            w = sl.stop - sl.start
            ps = psA.tile([P, 512], F32, tag="ps_wk", name=f"ps_wk{dh}{sl.start}")
            psw = ps[:, 0:w]
            for i, kc in enumerate(KC_ORDER):
                nc.tensor.matmul(psw, lhsT=wsp["ksw"][:, kc, dh * P:(dh + 1) * P],
                                 rhs=kbasis[:, kc, sl], start=(i == 0), stop=False)
            for ch in range(2):
                nc.tensor.matmul(psw, lhsT=wsp["kbw"][:, ch, dh * P:(dh + 1) * P],
                                 rhs=ksilu[:, ch, sl], start=False, stop=(ch == 1))
            nc.scalar.activation(wkT[:, dh, sl], psw, AF.Identity,
                                 bias=kbb[:, dh:dh + 1], scale=1.0)

        wk_part(0, slice(0, 512))
        wk_part(1, slice(0, 512))

        # ---- k chain half B ----
        chain_chunk(kT[:, :, 512:1024], kbasis, slice(512, 1024), 512)
        silu_chunk(kT[:, :, 512:1024], ksilu[:, :, 512:1024], 512)

        wk_part(0, slice(512, 1024))

        # ---- q chain ----
        qbasis = spool.tile([P, NKC, B], BF16, tag="qo_basis", bufs=1)
        qsilu = spool.tile([P, 2, B], BF16, bufs=1, name="qsilu")
        chain_chunk(qT, qbasis, slice(0, B), B)
        silu_chunk(qT, qsilu, B)

        # ---- v chain half A (fills ACT while wk-B runs on PE) ----
        vbasis = xpool.tile([P, NKC, S_C], BF16, name="vbasis")
        vsilu = xpool.tile([P, 2, S_C], BF16, name="vsilu")
        chain_chunk(vT[:, :, 0:512], vbasis, slice(0, 512), 512)
        silu_chunk(vT[:, :, 0:512], vsilu[:, :, 0:512], 512)

        wk_part(1, slice(512, 1024))

        # ---- wq spline (transposed out, fp16) ----
        wqT = attn.tile([P, 2, B], FP16)
        for dh in range(2):
            ps = psC.tile([P, B], F32, tag="ps_c", name=f"ps_wq{dh}")
            for i, kc in enumerate(KC_ORDER):
                nc.tensor.matmul(ps, lhsT=wsp["qsw"][:, kc, dh * P:(dh + 1) * P],
                                 rhs=qbasis[:, kc, :], start=(i == 0), stop=False)
            for ch in range(2):
                nc.tensor.matmul(ps, lhsT=wsp["qbw"][:, ch, dh * P:(dh + 1) * P],
                                 rhs=qsilu[:, ch, :], start=False, stop=(ch == 1))
            nc.vector.tensor_scalar(out=wqT[:, dh, :], in0=ps, scalar1=NORM,
                                    scalar2=qbbn[:, dh:dh + 1], op0=MM, op1=AD)

        # ---- block-diag wq4 for logits (Pool) ----
        for ch in range(2):
            for hq in range(4):
                rg = hq * 32
                nc.vector.tensor_copy(wq4[rg:rg + 32, ch, hq, :],
                                      wqT[rg:rg + 32, ch, :])

        # ---- logits + exp ----
        expt = {}

        def logits_sc(ch, sc):
            ps = psL.tile([P, 4 * B], F32, tag="ps_L", bufs=3, name=f"ps_L{ch}{sc}")
            nc.tensor.matmul(ps, lhsT=wkT[:, ch, sc * P:(sc + 1) * P],
                             rhs=wq4[:, ch, :, :].rearrange("p h b -> p (h b)"),
                             start=True, stop=True)
            e = attn.tile([P, 4, B], BF16, name=f"exp{ch}{sc}",
                          tag=f"exp{ch}", bufs=8)
            nc.scalar.activation(e, ps, AF.Exp)
            expt[(ch, sc)] = e

        for sc in range(8):
            logits_sc(0, sc)

        # ---- g spline (natural out) + 1+tanh(z/2) + gv ----
        g1t = attn.tile([B, DOUT], BF16)
        psg = psL.tile([B, DOUT], F32, tag="ps_gv", bufs=1, name="ps_g")
        for i, kc in enumerate(KC_ORDER):
            nc.tensor.matmul(psg, lhsT=qbasis[:, kc, :], rhs=wsp["gsw"][:, kc, :],
                             start=(i == 0), stop=False)
        for ch in range(2):
            nc.tensor.matmul(psg, lhsT=qsilu[:, ch, :], rhs=wsp["gbw"][:, ch, :],
                             start=False, stop=False)
        nc.tensor.matmul(psg, lhsT=ones_b, rhs=gbb, start=False, stop=True)
        gtmp = attn.tile([B, DOUT], BF16, name="gtanh")
        nc.scalar.activation(gtmp, psg, AF.Tanh, scale=0.5)
        nc.vector.tensor_scalar(out=g1t, in0=gtmp, scalar1=1.0, scalar2=0.5,
                                op0=AD, op1=MM)
        gv = attn.tile([B, DOUT], BF16)
        nc.vector.tensor_tensor(out=gv, in0=g1t, in1=vbb, op=MM)

        # ---- wv spline natural out (groups r = s-chunk of 128) ----
        def wv_part(r):

            rsl = slice(r * P, (r + 1) * P)
            ps = psC.tile([P, DOUT], F32, tag="ps_c", name=f"ps_wv{r}")
            for i, kc in enumerate(KC_ORDER):
                nc.tensor.matmul(ps, lhsT=vbasis[:, kc, rsl], rhs=wsp["vsw"][:, kc, :],
                                 start=(i == 0), stop=False)
            for ch in range(2):
                nc.tensor.matmul(ps, lhsT=vsilu[:, ch, rsl], rhs=wsp["vbw"][:, ch, :],
                                 start=False, stop=(ch == 1))
            if r < 4:
                nc.scalar.activation(wv_sb[:, r, :, 0:HD],
                                     ps.rearrange("p (h d) -> p h d", h=H), AF.Identity)
            else:
                nc.vector.tensor_copy(wv_sb[:, r, :, 0:HD],
                                      ps.rearrange("p (h d) -> p h d", h=H))

        for r in range(4):
            wv_part(r)
            logits_sc(1, r)

        # ---- v chain half B (ACT slot after the wv0-3 drains) ----
        chain_chunk(vT[:, :, 512:1024], vbasis, slice(512, 1024), 512)
        silu_chunk(vT[:, :, 512:1024], vsilu[:, :, 512:1024], 512)

        for r in range(4, 8):
            wv_part(r)
            logits_sc(1, r)

        # ---- PV (PSUM-accumulated across sc) ----
        pvps = psL.tile([B, H, HD + 1], F32, tag="ps_gv", bufs=1, name="ps_pv")
        nc.vector.memset(pvps, 0.0)
        for sc in range(8):
            for h in range(H):
                ch, hq = h // 4, h % 4
                nc.tensor.matmul(pvps[:, h, :],
                                 lhsT=expt[(ch, sc)][:, hq, :],
                                 rhs=wv_sb[:, sc, h, :],
                                 start=False, stop=(sc == 7),
                                 skip_group_check=True)
        opart = spool.tile([B, H, HD + 1], F32, bufs=1)
        nc.vector.tensor_copy(opart, pvps)
        nc.sync.dma_start(out=cc_in, in_=opart)

        if not mock_ar:
            nc.gpsimd.collective_compute("AllReduce", AD, replica_groups=AR_GROUP,
                                         ins=[cc_in], outs=[cc_out])

        # ---- combine + gate (natural layout), per-ch interleaved ----
        # mock mode reads cc_in directly: the loop then measures exactly the
        # two real DMA hops (cc_in write, result read); the AllReduce itself
        # (incl. its internal DRAM movement) is the AR_NS adder in test.py.
        oall = spool.tile([B, H, HD + 1], F32, bufs=1)
        nc.sync.dma_start(out=oall, in_=cc_in if mock_ar else cc_out)
        rl2 = spool.tile([B, H], F32, bufs=1)
        nc.vector.reciprocal(rl2, oall[:, :, HD])
        og = spool.tile([B, H, HD], F32, bufs=1)
        ogf = og.rearrange("b h d -> b (h d)")
        g1v = g1t.rearrange("b (h d) -> b h d", h=H)
        ogT = spool.tile([P, 2, B], F32, bufs=1)
        for ch in range(2):
            for hq in range(4):
                h = ch * 4 + hq
                nc.vector.scalar_tensor_tensor(
                    out=og[:, h, :], in0=oall[:, h, 0:HD], scalar=rl2[:, h:h + 1],
                    in1=g1v[:, h, :], op0=MM, op1=MM)
            nc.vector.tensor_tensor(out=ogf[:, ch * P:(ch + 1) * P],
                                    in0=ogf[:, ch * P:(ch + 1) * P],
                                    in1=gv[:, ch * P:(ch + 1) * P], op=AD)
            pt = psL.tile([P, P], F32, tag="ps_gv", bufs=1, name=f"ps_ogt{ch}")
            nc.tensor.transpose(pt, ogf[:, ch * P:(ch + 1) * P], ident)
            nc.vector.tensor_copy(ogT[:, ch, :], pt)

        # ---- output fastkan (ch-split: spline ch0 overlaps chain ch1) ----
        obasis = spool.tile([P, NKC, B], BF16, tag="obasis", bufs=1)
        osilu = spool.tile([P, 2, B], BF16, bufs=1, name="osilu")
        pso = psL.tile([B, OSH], F32, tag="ps_gv", bufs=1, name="ps_out")
        first = [True]

        def o_spline_ch(ch):
            for j in (0, 4, 1, 5, 2, 6, 3, 7):
                kc = j * 2 + ch
                nc.tensor.matmul(pso, lhsT=obasis[:, kc, :], rhs=wsp["osw"][:, kc, :],
                                 start=first[0], stop=False)
                first[0] = False
            nc.tensor.matmul(pso, lhsT=osilu[:, ch, :], rhs=wsp["obw"][:, ch, :],
                             start=False, stop=False)

        for ch in range(2):
            x1 = ogT[:, ch:ch + 1, :]
            chain_chunk(x1, obasis, slice(0, B), B, sq=nc.vector, chs=(ch,))
            silu_chunk(x1, osilu[:, ch:ch + 1, :], B)
            o_spline_ch(ch)
        nc.tensor.matmul(pso, lhsT=ones_b, rhs=obb, start=False, stop=True)
        out_sb = spool.tile([B, OSH], F32, bufs=1)
        nc.scalar.activation(out_sb, pso, AF.Identity)
        nc.sync.dma_start(out=d_out, in_=out_sb)

        if dbg:
            for nm, tile_, dt in (("dbg_wkT", wkT, FP16), ("dbg_wv", wv_sb, BF16),
                                  ("dbg_g1t", g1t, BF16), ("dbg_opart", opart, F32),
                                  ("dbg_wqT", wqT, FP16), ("dbg_kbasis", kbasis, BF16),
                                  ("dbg_e00", expt[(0, 0)], BF16),
                                  ("dbg_ogT", ogT, F32), ("dbg_oall", oall, F32)):
                dten = nc.dram_tensor(nm, list(tile_.shape), dt,
                                      kind="ExternalOutput").ap()
                nc.sync.dma_start(out=dten, in_=tile_)

        if loop_cm:
            loop_cm.__exit__(None, None, None)

    nc.compile()
    return nc


def xTb(x):
    return np.ascontiguousarray(
        x.T.reshape(2, P, -1).transpose(1, 0, 2).astype(ml_dtypes.bfloat16))


def prep_full(inp):
    """Host-side layout/constant refactorization shared by all cores."""
    C = np.exp(-GRID ** 2 / DEN ** 2)

    def spline_gm(w, scale_j):
        dout = w.shape[1]
        a = w.reshape(2, P, NG, dout).transpose(1, 2, 0, 3)   # p, j, ch, n
        a = a * scale_j[None, :, None, None]
        return np.ascontiguousarray(a.reshape(P, NKC, dout).astype(ml_dtypes.bfloat16))

    def base_h(w):
        dout = w.shape[1]
        a = (0.5 * w).reshape(2, P, dout).transpose(1, 0, 2)
        return np.ascontiguousarray(a.astype(ml_dtypes.bfloat16))

    def xT(x):
        # [N, 256] -> [128, 2, N] with (p, ch) laid out as dims ch*128+p
        return np.ascontiguousarray(
            x.T.reshape(2, P, -1).transpose(1, 0, 2).astype(np.float32))

    d = {}
    d["q"] = xT(inp["q"])
    for nm in ("q", "g", "k", "v"):
        d[f"{nm}_sw"] = spline_gm(inp[f"{nm}_sw"], C)
        d[f"{nm}_bw"] = base_h(inp[f"{nm}_bw"])
    d["q_bb"] = np.ascontiguousarray(
        (inp["q_bb"] * NORM).reshape(2, P).T.astype(np.float32))
    d["k_bb"] = np.ascontiguousarray(inp["k_bb"].reshape(2, P).T.astype(np.float32))
    d["g_bb"] = np.ascontiguousarray(
        inp["g_bb"].reshape(1, DOUT).astype(ml_dtypes.bfloat16))
    d["v_bb"] = np.ascontiguousarray(
        np.tile(inp["v_bb"].reshape(1, DOUT), (P, 1)).astype(ml_dtypes.bfloat16))
    return d


def shard_inputs(inp):
    full = prep_full(inp)
    C = np.exp(-GRID ** 2 / DEN ** 2)
    maps = []
    for c in range(8):
        m = dict(full)
        m["k"] = xTb(inp["k"][c * S_C:(c + 1) * S_C])
        m["v"] = xTb(inp["v"][c * S_C:(c + 1) * S_C])
        osw = inp["o_sw"][:, c * OSH:(c + 1) * OSH]
        a = osw.reshape(2, P, NG, OSH).transpose(1, 2, 0, 3) * C[None, :, None, None]
        m["o_sw"] = np.ascontiguousarray(a.reshape(P, NKC, OSH).astype(ml_dtypes.bfloat16))
        m["o_bw"] = np.ascontiguousarray(
            (0.5 * inp["o_bw"][:, c * OSH:(c + 1) * OSH]).reshape(2, P, OSH)
            .transpose(1, 0, 2).astype(ml_dtypes.bfloat16))
        m["o_bb"] = np.ascontiguousarray(
            inp["o_bb"][c * OSH:(c + 1) * OSH].reshape(1, OSH).astype(ml_dtypes.bfloat16))
        maps.append(m)
    return maps


def unshard_output(results):
    return np.hstack([results[c]["out"] for c in range(8)])


_CACHE = {}


def kernel(**inputs):
    """Full unsharded inputs -> full [128, 256] fp32 output."""
    from concourse.bass_utils import run_bass_kernel_spmd

    inp = {k: np.asarray(v) for k, v in inputs.items()}
    maps = shard_inputs(inp)
    if "nc" not in _CACHE:
        _CACHE["nc"] = build_program()
    res = run_bass_kernel_spmd(_CACHE["nc"], maps, core_ids=list(range(8)))
    return unshard_output(res.results).astype(np.float32)

